# revision 2
# baseline (speedup 1.0000x reference)
"""Trainium2 Bass kernel for the top-k ranking metric layer.

Computes, for each of 8192 users with 1000 candidates (1 positive + 999
negatives, channel 1 of a softmax pair):
  - in_top_k:  1.0 if the positive item ranks in the top 10 (after masking
               duplicate candidates to -inf), else 0.0
  - ndcg:      ln(2)/ln(rank+2) * in_top_k
  - weights:   1.0 unless all 999 negatives are duplicates

Rank identity (stable descending argsort): rank(item 0) equals
count_j(masked[j] > masked[0]).  Reformulated without any big-constant
masking:  cmp[j] = (l[j] > v0) > d[j]   with  v0 = l[0] - d[0]*2^60.
Case check against the reference big_neg masking:
  d0=0: cmp[j] = (l[j] > l[0]) and not d[j]   == ref exactly
  d0=1: v0 = -2^60 exactly (fl rounding), so l[j] > v0 always:
        cmp[j] = not d[j]                     == ref exactly
The same ACT pass that converts d to f32 accumulates sum(d) per user,
giving weights = (sum != 999) with no extra work.

Host-side marshaling (part of sharding): channel 0 of the logits pair is
never read by the reference, so only channel 1 ships to the device; the
0/1 dup mask ships as int8.  Per-core HBM traffic is 5.12MB.

DMA order on one FIFO HWDGE ring: all 8 dup tiles first (128KB each, all
landed by ~13% of the stream) so the ACT chain starts immediately, then
the 8 logits tiles.  Data-parallel across 8 NeuronCores: 1024 users/core.
"""

import numpy as np

_TRN_REPO = "/opt/trn_rl_repo"

NUM_CORES = 8
U = 8192                 # total users
ROW = 1000               # candidates per user
P = 128                  # SBUF partitions
U_CORE = U // NUM_CORES  # 1024 users per core
T = U_CORE // P          # 8 user-blocks per core
BIG = float(2.0 ** 60)   # dup-positive offset; fl(l0 - 2^60) == -2^60
LN2 = float(np.log(2.0))
TOP_K = 10.0
DUP_ALL = 999.0          # sum(dup) value meaning "999 dups"

_NC = None


def _ensure_path():
    import sys
    try:
        import concourse  # noqa: F401
    except ImportError:
        sys.path.insert(0, _TRN_REPO)


def _build_nc():
    _ensure_path()
    from contextlib import ExitStack

    import concourse.tile as tile
    from concourse import bacc, mybir

    AF = mybir.ActivationFunctionType
    OP = mybir.AluOpType
    f32 = mybir.dt.float32
    i8 = mybir.dt.int8

    nc = bacc.Bacc(
        "TRN2", target_bir_lowering=False, debug=False, num_devices=NUM_CORES
    )
    # channel-1 logits only, de-interleaved on the host
    ld = nc.dram_tensor("logits", [T, P, ROW], f32, kind="ExternalInput").ap()
    # dup mask as int8, per-tile layout [T, P, ROW]
    dd = nc.dram_tensor("dup", [T, P, ROW], i8, kind="ExternalInput").ap()
    outd = nc.dram_tensor("out", [P, 3 * T], f32, kind="ExternalOutput").ap()

    with tile.TileContext(nc) as tc, ExitStack() as ctx:
        lg = ctx.enter_context(tc.tile_pool(name="lg", bufs=1))
        dp = ctx.enter_context(tc.tile_pool(name="dp", bufs=1))
        ps = ctx.enter_context(tc.tile_pool(name="ps", bufs=T))
        cm = ctx.enter_context(tc.tile_pool(name="cm", bufs=3))
        sm = ctx.enter_context(tc.tile_pool(name="sm", bufs=4))
        st = ctx.enter_context(tc.tile_pool(name="st", bufs=1))

        cnt = st.tile([P, T], f32, tag="cnt")    # rank of item 0, per user
        dsm = st.tile([P, T], f32, tag="dsm")    # sum(dup), per user
        outt = st.tile([P, 3 * T], f32, tag="outt")

        lts = [
            lg.tile([P, ROW], f32, name=f"lt{t}", tag=f"lt{t}") for t in range(T)
        ]
        dts = [
            dp.tile([P, ROW], i8, name=f"dt{t}", tag=f"dt{t}") for t in range(T)
        ]
        # One FIFO ring (SP/sync HWDGE): dup tiles stream first so the ACT
        # dup->f32 chain starts as soon as tile 0's 128KB lands; logits
        # tiles follow in processing order.
        for t in range(T):
            nc.sync.dma_start(dts[t][:], dd[t])
        for t in range(T):
            nc.sync.dma_start(lts[t][:], ld[t])

        # Preload the Ln activation table during the DMA-bound phase so the
        # lazy ACT_TABLE_LOAD (~1.3us) doesn't land in the kernel tail.
        two = st.tile([P, 1], f32, tag="two")
        nc.vector.memset(two[:], 2.0)
        warm = st.tile([P, 1], f32, tag="warm")
        nc.scalar.activation(warm[:], two[:], AF.Ln, bias=two[:])

        for t in range(T):
            # df = d as f32 {0,1}; accum gives row-sum(dup) for weights
            df = ps.tile([P, ROW], f32, tag="df")
            nc.scalar.activation(
                df[:], dts[t][:], AF.Copy, accum_out=dsm[:, t : t + 1]
            )

            l1 = lts[t][:]
            # v0 = l[0] - d[0]*2^60  (== -2^60 exactly when d[0]=1)
            v0 = sm.tile([P, 1], f32, tag="v0")
            nc.vector.tensor_scalar(
                v0[:], df[:, 0:1], -BIG, l1[:, 0:1], op0=OP.mult, op1=OP.add
            )
            # cmp[j] = (l[j] > v0) > d[j] ; cnt = sum_j cmp[j]
            cmp = cm.tile([P, ROW], f32, tag="cmp")
            nc.vector.scalar_tensor_tensor(
                cmp[:],
                l1,
                v0[:],
                df[:],
                op0=OP.is_gt,
                op1=OP.is_gt,
                accum_out=cnt[:, t : t + 1],
            )

        # ---- finishing over [P, T] ----
        # weights = (sum(dup) != 999)
        nc.vector.tensor_scalar(
            outt[:, 2 * T : 3 * T], dsm[:], DUP_ALL, None, op0=OP.not_equal
        )
        # in_top_k = rank < 10
        nc.vector.tensor_scalar(outt[:, 0:T], cnt[:], TOP_K, None, op0=OP.is_lt)
        # ndcg = ln2 / ln(rank + 2) * in_top_k
        lnp = st.tile([P, T], f32, tag="lnp")
        nc.scalar.activation(lnp[:], cnt[:], AF.Ln, bias=two[:])
        rcp = st.tile([P, T], f32, tag="rcp")
        nc.vector.reciprocal(rcp[:], lnp[:])
        nc.vector.scalar_tensor_tensor(
            outt[:, T : 2 * T],
            rcp[:],
            LN2,
            outt[:, 0:T],
            op0=OP.mult,
            op1=OP.mult,
        )
        nc.sync.dma_start(outd, outt[:])

    nc.compile()
    return nc


def _get_nc():
    global _NC
    if _NC is None:
        _NC = _build_nc()
    return _NC


def _shard_inputs(logits, dup_mask):
    # channel 1 only: [U*ROW, 1, 2] -> [NUM_CORES, T, P, ROW]
    l1 = np.ascontiguousarray(
        np.asarray(logits, dtype=np.float32).reshape(U * ROW, 2)[:, 1]
    ).reshape(NUM_CORES, T, P, ROW)
    # dup as int8, same [NUM_CORES, T, P, ROW] layout
    d8 = np.asarray(dup_mask, dtype=np.int32).astype(np.int8).reshape(
        NUM_CORES, T, P, ROW
    )
    return [{"logits": l1[c], "dup": d8[c]} for c in range(NUM_CORES)]


def _unshard_outputs(per_core_outs):
    # out[p, t] holds user t*128+p of the core (col-blocks: topk | ndcg | wts)
    full = np.stack(per_core_outs)  # [C, P, 3T]
    in_top_k = np.ascontiguousarray(
        full[:, :, 0:T].transpose(0, 2, 1).reshape(U), dtype=np.float32
    )
    ndcg = np.ascontiguousarray(
        full[:, :, T : 2 * T].transpose(0, 2, 1).reshape(U), dtype=np.float32
    )
    wts = np.ascontiguousarray(
        full[:, :, 2 * T : 3 * T].transpose(0, 2, 1).reshape(U), dtype=np.float32
    )
    return in_top_k, ndcg, wts


def _run(logits, dup_mask, trace=False, **kwargs):
    """Run on hardware; returns ((in_top_k, ndcg, weights), BassKernelResults)."""
    _ensure_path()
    from concourse.bass_utils import run_bass_kernel_spmd

    nc = _get_nc()
    in_maps = _shard_inputs(logits, dup_mask)
    res = run_bass_kernel_spmd(
        nc, in_maps, core_ids=list(range(NUM_CORES)), trace=trace, **kwargs
    )
    outs = [res.results[c]["out"] for c in range(NUM_CORES)]
    return _unshard_outputs(outs), res


def kernel(logits, dup_mask):
    (in_top_k, ndcg, wts), _ = _run(logits, dup_mask)
    return in_top_k, ndcg, wts


# revision 3
# speedup vs baseline: 1.0626x; 1.0626x over previous
"""Trainium2 Bass kernel for the top-k ranking metric layer.

Computes, for each of 8192 users with 1000 candidates (1 positive + 999
negatives, channel 1 of a softmax pair):
  - in_top_k:  1.0 if the positive item ranks in the top 10 (after masking
               duplicate candidates to -inf), else 0.0
  - ndcg:      ln(2)/ln(rank+2) * in_top_k
  - weights:   1.0 unless all 999 negatives are duplicates

Rank identity (stable descending argsort): rank(item 0) equals
count_j(masked[j] > masked[0]).  Reformulated without big-constant
masking:  cmp[j] = (l[j] > v0) > d[j]   with  v0 = l[0] - d[0]*2^60.
Case check against the reference big_neg masking:
  d0=0: cmp[j] = (l[j] > l[0]) and not d[j]   == ref exactly
  d0=1: v0 = -2^60 exactly (fl rounding), so l[j] > v0 always:
        cmp[j] = not d[j]                     == ref exactly
The ACT pass that converts d to f32 also accumulates sum(d) per user,
giving weights = (sum != 999) for free.

Pipeline layout: one FIFO HWDGE ring (sync). The two dup halves stream
first (so the ACT chain starts at ~9us), then the 8 logits tiles as 16
half-tile DMAs so the DVE compare chain tracks arrivals at [P,500]
granularity.  v0 per tile is computed on the otherwise-idle Pool engine.
Data-parallel across 8 NeuronCores: 1024 users per core.
"""

import numpy as np

_TRN_REPO = "/opt/trn_rl_repo"

NUM_CORES = 8
U = 8192                 # total users
ROW = 1000               # candidates per user
HALF = ROW // 2
P = 128                  # SBUF partitions
U_CORE = U // NUM_CORES  # 1024 users per core
T = U_CORE // P          # 8 user-blocks per core
DROW = 1024              # dup row padded to 1024 for 4B-aligned slices
BIG = float(2.0 ** 60)   # dup-positive offset; fl(l0 - 2^60) == -2^60
LN2 = float(np.log(2.0))
TOP_K = 10.0
DUP_ALL = 999.0          # sum(dup) value meaning "999 dups"

_NC = None


def _ensure_path():
    import sys
    try:
        import concourse  # noqa: F401
    except ImportError:
        sys.path.insert(0, _TRN_REPO)


def _build_nc():
    _ensure_path()
    from contextlib import ExitStack

    import concourse.tile as tile
    from concourse import bacc, mybir

    AF = mybir.ActivationFunctionType
    OP = mybir.AluOpType
    f32 = mybir.dt.float32
    i8 = mybir.dt.int8

    nc = bacc.Bacc(
        "TRN2", target_bir_lowering=False, debug=False, num_devices=NUM_CORES
    )
    # channel-1 logits only, de-interleaved on the host
    ld = nc.dram_tensor("logits", [T, P, ROW], f32, kind="ExternalInput").ap()
    # dup mask as int8, host-transposed to [P, T*DROW] (zero-padded rows)
    dd = nc.dram_tensor("dup", [P, T * DROW], i8, kind="ExternalInput").ap()
    outd = nc.dram_tensor("out", [P, 3 * T], f32, kind="ExternalOutput").ap()

    with tile.TileContext(nc) as tc, ExitStack() as ctx:
        lg = ctx.enter_context(tc.tile_pool(name="lg", bufs=1))
        dp = ctx.enter_context(tc.tile_pool(name="dp", bufs=1))
        ps = ctx.enter_context(tc.tile_pool(name="ps", bufs=T))
        cm = ctx.enter_context(tc.tile_pool(name="cm", bufs=3))
        sm = ctx.enter_context(tc.tile_pool(name="sm", bufs=T))
        st = ctx.enter_context(tc.tile_pool(name="st", bufs=1))

        cnta = st.tile([P, T], f32, tag="cnta")  # rank partial, first half
        cntb = st.tile([P, T], f32, tag="cntb")  # rank partial, second half
        cnt = st.tile([P, T], f32, tag="cnt")    # rank of item 0, per user
        dsm = st.tile([P, T], f32, tag="dsm")    # sum(dup), per user
        outt = st.tile([P, 3 * T], f32, tag="outt")

        lts = [
            lg.tile([P, ROW], f32, name=f"lt{t}", tag=f"lt{t}") for t in range(T)
        ]
        H = T // 2
        dup_a = dp.tile([P, H * DROW], i8, name="dup_a", tag="dup_a")
        dup_b = dp.tile([P, H * DROW], i8, name="dup_b", tag="dup_b")
        # One FIFO ring: dup halves first (ACT chain unblocks early), then
        # logits as half-tile DMAs for fine-grained arrival tracking.
        nc.sync.dma_start(dup_a[:], dd[:, 0 : H * DROW])
        nc.sync.dma_start(dup_b[:], dd[:, H * DROW : T * DROW])
        for t in range(T):
            nc.sync.dma_start(lts[t][:, 0:HALF], ld[t][:, 0:HALF])
            nc.sync.dma_start(lts[t][:, HALF:ROW], ld[t][:, HALF:ROW])

        # Preload the Ln activation table during the DMA-bound phase so the
        # lazy ACT_TABLE_LOAD (~1.3us) doesn't land in the kernel tail.
        two = st.tile([P, 1], f32, tag="two")
        nc.vector.memset(two[:], 2.0)
        warm = st.tile([P, 1], f32, tag="warm")
        nc.scalar.activation(warm[:], two[:], AF.Ln, bias=two[:])

        def dup_slice(t):
            half = dup_a if t < H else dup_b
            tt = t % H
            return half[:, tt * DROW : tt * DROW + ROW]

        dfs = []
        for t in range(T):
            # df = d as f32 {0,1}; accum gives row-sum(dup) for weights
            df = ps.tile([P, ROW], f32, tag="df")
            nc.scalar.activation(
                df[:], dup_slice(t), AF.Copy, accum_out=dsm[:, t : t + 1]
            )
            dfs.append(df)
            # v0 = l[0] - d[0]*2^60 on the idle Pool engine
            v0 = sm.tile([P, 1], f32, tag=f"v0_{t}")
            nc.gpsimd.tensor_scalar(
                v0[:], df[:, 0:1], -BIG, lts[t][:, 0:1], op0=OP.mult, op1=OP.add
            )
            # cmp[j] = (l[j] > v0) > d[j], accumulated per half
            cmp = cm.tile([P, ROW], f32, tag="cmp")
            nc.vector.scalar_tensor_tensor(
                cmp[:, 0:HALF],
                lts[t][:, 0:HALF],
                v0[:],
                df[:, 0:HALF],
                op0=OP.is_gt,
                op1=OP.is_gt,
                accum_out=cnta[:, t : t + 1],
            )
            nc.vector.scalar_tensor_tensor(
                cmp[:, HALF:ROW],
                lts[t][:, HALF:ROW],
                v0[:],
                df[:, HALF:ROW],
                op0=OP.is_gt,
                op1=OP.is_gt,
                accum_out=cntb[:, t : t + 1],
            )

        # ---- finishing over [P, T] ----
        # weights = (sum(dup) != 999); dsm ready before the last logits tile
        nc.vector.tensor_scalar(
            outt[:, 2 * T : 3 * T], dsm[:], DUP_ALL, None, op0=OP.not_equal
        )
        nc.vector.tensor_tensor(cnt[:], cnta[:], cntb[:], op=OP.add)
        # in_top_k = rank < 10
        nc.vector.tensor_scalar(outt[:, 0:T], cnt[:], TOP_K, None, op0=OP.is_lt)
        # ndcg = ln2 / ln(rank + 2) * in_top_k
        lnp = st.tile([P, T], f32, tag="lnp")
        nc.scalar.activation(lnp[:], cnt[:], AF.Ln, bias=two[:])
        rcp = st.tile([P, T], f32, tag="rcp")
        nc.vector.reciprocal(rcp[:], lnp[:])
        nc.vector.scalar_tensor_tensor(
            outt[:, T : 2 * T],
            rcp[:],
            LN2,
            outt[:, 0:T],
            op0=OP.mult,
            op1=OP.mult,
        )
        nc.sync.dma_start(outd, outt[:])

    nc.compile()
    return nc


def _get_nc():
    global _NC
    if _NC is None:
        _NC = _build_nc()
    return _NC


def _shard_inputs(logits, dup_mask):
    # channel 1 only: [U*ROW, 1, 2] -> [NUM_CORES, T, P, ROW]
    l1 = np.ascontiguousarray(
        np.asarray(logits, dtype=np.float32).reshape(U * ROW, 2)[:, 1]
    ).reshape(NUM_CORES, T, P, ROW)
    # dup as int8, padded rows of DROW, transposed to [NUM_CORES, P, T*DROW]
    dm = np.asarray(dup_mask, dtype=np.int32).reshape(NUM_CORES, T, P, ROW)
    d8 = np.zeros((NUM_CORES, T, P, DROW), dtype=np.int8)
    d8[..., :ROW] = dm.astype(np.int8)
    d8 = np.ascontiguousarray(d8.transpose(0, 2, 1, 3)).reshape(
        NUM_CORES, P, T * DROW
    )
    return [{"logits": l1[c], "dup": d8[c]} for c in range(NUM_CORES)]


def _unshard_outputs(per_core_outs):
    # out[p, t] holds user t*128+p of the core (col-blocks: topk | ndcg | wts)
    full = np.stack(per_core_outs)  # [C, P, 3T]
    in_top_k = np.ascontiguousarray(
        full[:, :, 0:T].transpose(0, 2, 1).reshape(U), dtype=np.float32
    )
    ndcg = np.ascontiguousarray(
        full[:, :, T : 2 * T].transpose(0, 2, 1).reshape(U), dtype=np.float32
    )
    wts = np.ascontiguousarray(
        full[:, :, 2 * T : 3 * T].transpose(0, 2, 1).reshape(U), dtype=np.float32
    )
    return in_top_k, ndcg, wts


def _run(logits, dup_mask, trace=False, **kwargs):
    """Run on hardware; returns ((in_top_k, ndcg, weights), BassKernelResults)."""
    _ensure_path()
    from concourse.bass_utils import run_bass_kernel_spmd

    nc = _get_nc()
    in_maps = _shard_inputs(logits, dup_mask)
    res = run_bass_kernel_spmd(
        nc, in_maps, core_ids=list(range(NUM_CORES)), trace=trace, **kwargs
    )
    outs = [res.results[c]["out"] for c in range(NUM_CORES)]
    return _unshard_outputs(outs), res


def kernel(logits, dup_mask):
    (in_top_k, ndcg, wts), _ = _run(logits, dup_mask)
    return in_top_k, ndcg, wts
